# revision 14
# baseline (speedup 1.0000x reference)
"""DepthAwareConv2d Trainium2 kernel (bf16 shift-conv).

Math: the reference's depth-modulated im2col GEMM is exactly
    out = conv2d(x * depth, weight, stride=1, pad=1) + bias
(depth broadcasts over channels; unfold(x)*unfold(depth) = unfold(x*depth)).

Sharding (8 cores): data-parallel over N (4 images) x spatial-parallel over
image row halves.  Core cid handles n = cid//2, row half = cid%2, computing
all 256 output channels for its 64 output rows.  The host premultiplies
y = x*depth (0.02% of the module FLOPs), rounds to bf16, and bakes the
halo/zero borders into each core's shard, so the device program is pure
conv: DMA in, 288 bf16 matmuls, bias-activation, DMA out.

Why bf16: on this hardware fp32r matmuls run at 244.3ns per 512-out
instruction (the 4-byte self-loading weight path costs ~31ns extra per
matmul), while bf16 matmuls run at 216.0ns with the LDWEIGHTS overlapped
on its own track.  fp8 DoubleRow also runs 216ns but carries 2 k-tiles;
reaching 2e-2 accuracy needs 3 k-tile products per tap (hi/lo splits),
which is 14 instrs/block vs bf16's 9 -- bf16 wins.  Accuracy with bf16
inputs and bf16 output is ~3e-3, comfortably inside the 2e-2 gate.

Per-core device kernel:
  1. PE warm-up on a zeros tile bridges the input-DMA window so the HAM
     clock gate reaches 2.4GHz before the real train starts.
  2. Shift-conv: per 4-row output block and 128-wide out-channel block,
     9 accumulating bf16 matmuls (stationary = 128x128 weight tap, moving
     = shifted 4x128 window of the padded image, free dim 512 = one PSUM
     bank) -> 32 blocks, back-to-back on the PE.
  3. ScalarE Identity(+bias) PSUM->SBUF in bf16, DMA out per block.
"""

import numpy as np
import ml_dtypes

import concourse.bass as bass
import concourse.mybir as mybir
import concourse.tile as tile
from concourse import bacc
from concourse.bass_utils import run_bass_kernel_spmd

N, C, O, H, W = 4, 128, 256, 128, 128
HSH = H // 2  # output rows per core
HIN = HSH + 2  # input rows per core incl. halo/zero rows
W2 = W + 2  # padded columns (zero borders baked in on host)
O2 = O // 2
NCORES = 8
F32 = mybir.dt.float32
BF16 = mybir.dt.bfloat16
NPBF16 = ml_dtypes.bfloat16
ACT_IDENT = mybir.ActivationFunctionType.Identity

RB = 4  # output rows per matmul tile (free dim RB*W = 512, one PSUM bank)
# warm-ups bridge from PE-ready (~8.0us) to input-data-ready (~10.4us) at the
# mid p-state rate (~107ns per 128-free matmul); the real train then starts
# near full clock
NWARM = 24
# image rows per load chunk; a small first chunk lets the first matmul
# block (rows 0..5) start as early as possible
CHUNKS = ((0, 8), (8, 16), (16, 28), (28, 40), (40, 52), (52, 66))

_CACHE = {}


def build_nc():
    nc = bacc.Bacc("TRN2", target_bir_lowering=False, debug=False, num_devices=NCORES)
    ys = nc.declare_dram_parameter("ys", [C, HIN, W2], BF16, isOutput=False)
    wt = nc.declare_dram_parameter("wt", [C, 2, 9, O2], BF16, isOutput=False)
    bb = nc.declare_dram_parameter("bb", [O2, 2], F32, isOutput=False)
    out = nc.declare_dram_parameter("out", [O, HSH, W], BF16, isOutput=True)

    with tile.TileContext(nc) as tc:
        with (
            tc.tile_pool(name="big", bufs=1) as big,
            tc.tile_pool(name="wp", bufs=1) as wp,
            tc.tile_pool(name="op", bufs=4) as op,
            tc.tile_pool(name="pp", bufs=8, space="PSUM") as pp,
        ):
            ysb = big.tile([C, HIN, W2], BF16)
            wsb = wp.tile([C, 2, 9, O2], BF16)
            bsb = wp.tile([O2, 2], F32)  # bsb[o, ob] = bias[ob*128 + o]
            # warm-up operand: smallest possible memset so the PE can start
            # churning as early as the start barrier allows
            z = wp.tile([C, 128], BF16)
            nc.vector.memset(z, 0.0)

            # PE warm-up: zero matmuls bridge the input-DMA window so the
            # clock is at full rate when the real train starts (a stall-free
            # later start beats an earlier starved start).
            warm = pp.tile([O2, 128], F32, tag="ps")
            for _ in range(NWARM):
                nc.tensor.matmul(warm, z, z, start=True, stop=True)

            # first image chunk on the Activation DGE queue, in parallel with
            # the weights on the sync queue: both land ~10.3us, right at the
            # p-state full-clock boundary
            r0, r1 = CHUNKS[0]
            nc.scalar.dma_start(out=ysb[:, r0:r1, :], in_=ys[:, r0:r1, :])
            nc.sync.dma_start(out=wsb[:, 0], in_=wt[:, 0])
            nc.sync.dma_start(out=wsb[:, 1], in_=wt[:, 1])
            nc.sync.dma_start(out=bsb, in_=bb.ap())
            for r0, r1 in CHUNKS[1:]:
                nc.sync.dma_start(out=ysb[:, r0:r1, :], in_=ys[:, r0:r1, :])

            # row blocks: RB rows each, except the last RB rows go as two
            # 2-row blocks so the final activation + output DMA (the kernel
            # tail after the last matmul) cover half the data
            rblocks = [(rb, RB) for rb in range(0, HSH - RB, RB)]
            rblocks += [(HSH - RB, 2), (HSH - 2, 1), (HSH - 1, 1)]
            for rb, nr in rblocks:
                for ob in range(2):
                    ps = pp.tile([O2, nr, W], F32, tag="ps", name=f"ps{rb}_{ob}")
                    for p in range(9):
                        i, j = divmod(p, 3)
                        nc.tensor.matmul(
                            ps,
                            wsb[:, ob, p],
                            ysb[:, rb + i : rb + i + nr, j : j + W],
                            start=(p == 0),
                            stop=(p == 8),
                        )
                    osb = op.tile([O2, nr, W], BF16, tag="osb", name=f"osb{rb}_{ob}")
                    nc.scalar.activation(
                        out=osb,
                        in_=ps,
                        func=ACT_IDENT,
                        bias=bsb[:, ob : ob + 1],
                        scale=1.0,
                    )
                    # per-ob output DMA: each half ships while the other
                    # half's matmuls still run, shortening the kernel tail
                    nc.sync.dma_start(
                        out=out[ob * O2 : (ob + 1) * O2, rb : rb + nr, :],
                        in_=osb,
                    )

    nc.compile()
    return nc


def _get_nc():
    if "nc" not in _CACHE:
        _CACHE["nc"] = build_nc()
    return _CACHE["nc"]


def make_in_maps(x, depth, weight, bias):
    x = np.asarray(x, np.float32)
    depth = np.asarray(depth, np.float32)
    weight = np.asarray(weight, np.float32)
    bias = np.asarray(bias, np.float32)
    y = x * depth  # (N, C, H, W) fp32; bf16-rounded below
    # (O, C, 3, 3) -> (C, ob, tap=i*3+j, o) with o = local index in the
    # 128-wide out-channel half ob
    wt9 = np.ascontiguousarray(
        np.transpose(weight.reshape(2, O2, C, 3, 3), (2, 0, 3, 4, 1)).reshape(
            C, 2, 9, O2
        )
    ).astype(NPBF16)
    bb = np.ascontiguousarray(bias.reshape(2, O2).T)
    in_maps = []
    for cid in range(NCORES):
        n, hh = divmod(cid, 2)
        ysh = np.zeros((C, HIN, W2), NPBF16)
        if hh == 0:
            ysh[:, 1:, 1 : W + 1] = y[n, :, : HSH + 1].astype(NPBF16)
        else:
            ysh[:, :-1, 1 : W + 1] = y[n, :, HSH - 1 :].astype(NPBF16)
        in_maps.append({"ys": ysh, "wt": wt9, "bb": bb})
    return in_maps


def gather_out(results):
    out = np.empty((N, O, H, W), np.float32)
    for cid in range(NCORES):
        n, hh = divmod(cid, 2)
        out[n, :, hh * HSH : (hh + 1) * HSH] = results[cid]["out"].astype(np.float32)
    return out


def kernel(x, depth, camera_params, weight, bias):
    nc = _get_nc()
    in_maps = make_in_maps(x, depth, weight, bias)
    res = run_bass_kernel_spmd(nc, in_maps, list(range(NCORES)))
    return gather_out(res.results)


# revision 15
# speedup vs baseline: 1.0126x; 1.0126x over previous
"""DepthAwareConv2d Trainium2 kernel (bf16 shift-conv).

Math: the reference's depth-modulated im2col GEMM is exactly
    out = conv2d(x * depth, weight, stride=1, pad=1) + bias
(depth broadcasts over channels; unfold(x)*unfold(depth) = unfold(x*depth)).

Sharding (8 cores): data-parallel over N (4 images) x spatial-parallel over
image row halves.  Core cid handles n = cid//2, row half = cid%2, computing
all 256 output channels for its 64 output rows.  The host premultiplies
y = x*depth (0.02% of the module FLOPs), rounds to bf16, and bakes the
halo/zero borders into each core's shard, so the device program is pure
conv: DMA in, 288 bf16 matmuls, bias-activation, DMA out.

Why bf16: on this hardware fp32r matmuls run at 244.3ns per 512-out
instruction (the 4-byte self-loading weight path costs ~31ns extra per
matmul), while bf16 matmuls run at 216.0ns with the LDWEIGHTS overlapped
on its own track.  fp8 DoubleRow also runs 216ns but carries 2 k-tiles;
reaching 2e-2 accuracy needs 3 k-tile products per tap (hi/lo splits),
which is 14 instrs/block vs bf16's 9 -- bf16 wins.  Accuracy with bf16
inputs and bf16 output is ~3e-3, comfortably inside the 2e-2 gate.

Per-core device kernel:
  1. PE warm-up on a zeros tile bridges the input-DMA window so the HAM
     clock gate reaches 2.4GHz before the real train starts.
  2. Shift-conv: per 4-row output block and 128-wide out-channel block,
     9 accumulating bf16 matmuls (stationary = 128x128 weight tap, moving
     = shifted 4x128 window of the padded image, free dim 512 = one PSUM
     bank) -> 32 blocks, back-to-back on the PE.
  3. ScalarE Identity(+bias) PSUM->SBUF in bf16, DMA out per block.
"""

import numpy as np
import ml_dtypes

import concourse.bass as bass
import concourse.mybir as mybir
import concourse.tile as tile
from concourse import bacc
from concourse.bass_utils import run_bass_kernel_spmd

N, C, O, H, W = 4, 128, 256, 128, 128
HSH = H // 2  # output rows per core
HIN = HSH + 2  # input rows per core incl. halo/zero rows
W2 = W + 2  # padded columns (zero borders baked in on host)
O2 = O // 2
NCORES = 8
F32 = mybir.dt.float32
BF16 = mybir.dt.bfloat16
NPBF16 = ml_dtypes.bfloat16
ACT_IDENT = mybir.ActivationFunctionType.Identity

RB = 4  # output rows per matmul tile (free dim RB*W = 512, one PSUM bank)
# warm-ups bridge from PE-ready (~7.6us) to input-data-ready (~11.7us) at the
# mid p-state rate (~107ns per 128-free matmul); covering the whole window
# keeps the clock ramp alive so the real train starts at full rate
NWARM = 38
# image rows per load chunk; a small first chunk lets the first matmul
# block (rows 0..5) start as early as possible
CHUNKS = ((0, 8), (8, 16), (16, 28), (28, 40), (40, 52), (52, 66))

_CACHE = {}


def build_nc():
    nc = bacc.Bacc("TRN2", target_bir_lowering=False, debug=False, num_devices=NCORES)
    ys = nc.declare_dram_parameter("ys", [C, HIN, W2], BF16, isOutput=False)
    wt = nc.declare_dram_parameter("wt", [C, 2, 9, O2], BF16, isOutput=False)
    bb = nc.declare_dram_parameter("bb", [O2, 2], F32, isOutput=False)
    out = nc.declare_dram_parameter("out", [O, HSH, W], BF16, isOutput=True)

    with tile.TileContext(nc) as tc:
        with (
            tc.tile_pool(name="big", bufs=1) as big,
            tc.tile_pool(name="wp", bufs=1) as wp,
            tc.tile_pool(name="op", bufs=4) as op,
            tc.tile_pool(name="pp", bufs=8, space="PSUM") as pp,
        ):
            ysb = big.tile([C, HIN, W2], BF16)
            wsb = wp.tile([C, 2, 9, O2], BF16)
            bsb = wp.tile([O2, 2], F32)  # bsb[o, ob] = bias[ob*128 + o]
            # warm-up operand: smallest possible memset so the PE can start
            # churning as early as the start barrier allows
            z = wp.tile([C, 128], BF16)
            nc.vector.memset(z, 0.0)

            # PE warm-up: zero matmuls bridge the input-DMA window so the
            # clock is at full rate when the real train starts (a stall-free
            # later start beats an earlier starved start).
            warm = pp.tile([O2, 128], F32, tag="ps")
            for _ in range(NWARM):
                nc.tensor.matmul(warm, z, z, start=True, stop=True)

            # first image chunk on the Activation DGE queue, in parallel with
            # the weights on the sync queue: both land ~10.3us, right at the
            # p-state full-clock boundary
            r0, r1 = CHUNKS[0]
            nc.scalar.dma_start(out=ysb[:, r0:r1, :], in_=ys[:, r0:r1, :])
            nc.sync.dma_start(out=wsb[:, 0], in_=wt[:, 0])
            nc.sync.dma_start(out=wsb[:, 1], in_=wt[:, 1])
            nc.sync.dma_start(out=bsb, in_=bb.ap())
            for r0, r1 in CHUNKS[1:]:
                nc.sync.dma_start(out=ysb[:, r0:r1, :], in_=ys[:, r0:r1, :])

            # row blocks: RB rows each, except the last RB rows go as two
            # 2-row blocks so the final activation + output DMA (the kernel
            # tail after the last matmul) cover half the data
            rblocks = [(rb, RB) for rb in range(0, HSH - RB, RB)]
            rblocks += [(HSH - RB, 2), (HSH - 2, 1), (HSH - 1, 1)]
            for rb, nr in rblocks:
                for ob in range(2):
                    ps = pp.tile([O2, nr, W], F32, tag="ps", name=f"ps{rb}_{ob}")
                    for p in range(9):
                        i, j = divmod(p, 3)
                        nc.tensor.matmul(
                            ps,
                            wsb[:, ob, p],
                            ysb[:, rb + i : rb + i + nr, j : j + W],
                            start=(p == 0),
                            stop=(p == 8),
                        )
                    osb = op.tile([O2, nr, W], BF16, tag="osb", name=f"osb{rb}_{ob}")
                    nc.scalar.activation(
                        out=osb,
                        in_=ps,
                        func=ACT_IDENT,
                        bias=bsb[:, ob : ob + 1],
                        scale=1.0,
                    )
                    # per-ob output DMA: each half ships while the other
                    # half's matmuls still run, shortening the kernel tail
                    nc.sync.dma_start(
                        out=out[ob * O2 : (ob + 1) * O2, rb : rb + nr, :],
                        in_=osb,
                    )

    nc.compile()
    return nc


def _get_nc():
    if "nc" not in _CACHE:
        _CACHE["nc"] = build_nc()
    return _CACHE["nc"]


def make_in_maps(x, depth, weight, bias):
    x = np.asarray(x, np.float32)
    depth = np.asarray(depth, np.float32)
    weight = np.asarray(weight, np.float32)
    bias = np.asarray(bias, np.float32)
    y = x * depth  # (N, C, H, W) fp32; bf16-rounded below
    # (O, C, 3, 3) -> (C, ob, tap=i*3+j, o) with o = local index in the
    # 128-wide out-channel half ob
    wt9 = np.ascontiguousarray(
        np.transpose(weight.reshape(2, O2, C, 3, 3), (2, 0, 3, 4, 1)).reshape(
            C, 2, 9, O2
        )
    ).astype(NPBF16)
    bb = np.ascontiguousarray(bias.reshape(2, O2).T)
    in_maps = []
    for cid in range(NCORES):
        n, hh = divmod(cid, 2)
        ysh = np.zeros((C, HIN, W2), NPBF16)
        if hh == 0:
            ysh[:, 1:, 1 : W + 1] = y[n, :, : HSH + 1].astype(NPBF16)
        else:
            ysh[:, :-1, 1 : W + 1] = y[n, :, HSH - 1 :].astype(NPBF16)
        in_maps.append({"ys": ysh, "wt": wt9, "bb": bb})
    return in_maps


def gather_out(results):
    out = np.empty((N, O, H, W), np.float32)
    for cid in range(NCORES):
        n, hh = divmod(cid, 2)
        out[n, :, hh * HSH : (hh + 1) * HSH] = results[cid]["out"].astype(np.float32)
    return out


def kernel(x, depth, camera_params, weight, bias):
    nc = _get_nc()
    in_maps = make_in_maps(x, depth, weight, bias)
    res = run_bass_kernel_spmd(nc, in_maps, list(range(NCORES)))
    return gather_out(res.results)


# revision 16
# speedup vs baseline: 1.0318x; 1.0189x over previous
"""DepthAwareConv2d Trainium2 kernel (bf16 shift-conv).

Math: the reference's depth-modulated im2col GEMM is exactly
    out = conv2d(x * depth, weight, stride=1, pad=1) + bias
(depth broadcasts over channels; unfold(x)*unfold(depth) = unfold(x*depth)).

Sharding (8 cores): data-parallel over N (4 images) x spatial-parallel over
image row halves.  Core cid handles n = cid//2, row half = cid%2, computing
all 256 output channels for its 64 output rows.  The host premultiplies
y = x*depth (0.02% of the module FLOPs), rounds to bf16, and bakes the
halo/zero borders into each core's shard, so the device program is pure
conv: DMA in, 288 bf16 matmuls, bias-activation, DMA out.

Why bf16: on this hardware fp32r matmuls run at 244.3ns per 512-out
instruction (the 4-byte self-loading weight path costs ~31ns extra per
matmul), while bf16 matmuls run at 216.0ns with the LDWEIGHTS overlapped
on its own track.  fp8 DoubleRow also runs 216ns but carries 2 k-tiles;
reaching 2e-2 accuracy needs 3 k-tile products per tap (hi/lo splits),
which is 14 instrs/block vs bf16's 9 -- bf16 wins.  Accuracy with bf16
inputs and bf16 output is ~3e-3, comfortably inside the 2e-2 gate.

Per-core device kernel:
  1. PE warm-up on a zeros tile bridges the input-DMA window so the HAM
     clock gate reaches 2.4GHz before the real train starts.
  2. Shift-conv: per 4-row output block and 128-wide out-channel block,
     9 accumulating bf16 matmuls (stationary = 128x128 weight tap, moving
     = shifted 4x128 window of the padded image, free dim 512 = one PSUM
     bank) -> 32 blocks, back-to-back on the PE.
  3. ScalarE Identity(+bias) PSUM->SBUF in bf16, DMA out per block.
"""

import numpy as np
import ml_dtypes

import concourse.bass as bass
import concourse.mybir as mybir
import concourse.tile as tile
from concourse import bacc
from concourse.bass_utils import run_bass_kernel_spmd

N, C, O, H, W = 4, 128, 256, 128, 128
HSH = H // 2  # output rows per core
HIN = HSH + 2  # input rows per core incl. halo/zero rows
W2 = W + 2  # padded columns (zero borders baked in on host)
O2 = O // 2
NCORES = 8
F32 = mybir.dt.float32
BF16 = mybir.dt.bfloat16
NPBF16 = ml_dtypes.bfloat16
ACT_IDENT = mybir.ActivationFunctionType.Identity

RB = 4  # output rows per matmul tile (free dim RB*W = 512, one PSUM bank)
# warm-ups bridge from PE-ready (~7.6us) to input-data-ready (~11.7us) at the
# mid p-state rate (~107ns per 128-free matmul); covering the whole window
# keeps the clock ramp alive so the real train starts at full rate
NWARM = 38
# image rows per load chunk; a small first chunk lets the first matmul
# block (rows 0..5) start as early as possible
CHUNKS = ((0, 8), (8, 16), (16, 28), (28, 40), (40, 52), (52, 66))

_CACHE = {}


def build_nc():
    nc = bacc.Bacc("TRN2", target_bir_lowering=False, debug=False, num_devices=NCORES)
    ys = nc.declare_dram_parameter("ys", [C, HIN, W2], BF16, isOutput=False)
    wt = nc.declare_dram_parameter("wt", [C, 2, 9, O2], BF16, isOutput=False)
    bb = nc.declare_dram_parameter("bb", [O2, 2], F32, isOutput=False)
    out = nc.declare_dram_parameter("out", [O, HSH, W], BF16, isOutput=True)

    with tile.TileContext(nc) as tc:
        with (
            tc.tile_pool(name="big", bufs=1) as big,
            tc.tile_pool(name="wp", bufs=1) as wp,
            tc.tile_pool(name="op", bufs=4) as op,
            tc.tile_pool(name="pp", bufs=8, space="PSUM") as pp,
        ):
            ysb = big.tile([C, HIN, W2], BF16)
            wsb = wp.tile([C, 2, 9, O2], BF16)
            bsb = wp.tile([O2, 2], F32)  # bsb[o, ob] = bias[ob*128 + o]
            # warm-up operand: smallest possible memset so the PE can start
            # churning as early as the start barrier allows
            z = wp.tile([C, 128], BF16)
            nc.vector.memset(z, 0.0)

            # PE warm-up: zero matmuls bridge the input-DMA window so the
            # clock is at full rate when the real train starts (a stall-free
            # later start beats an earlier starved start).
            warm = pp.tile([O2, 128], F32, tag="ps")
            for _ in range(NWARM):
                nc.tensor.matmul(warm, z, z, start=True, stop=True)

            # first image chunk on the Activation DGE queue, in parallel with
            # the weights on the sync queue: both land ~10.3us, right at the
            # p-state full-clock boundary
            r0, r1 = CHUNKS[0]
            nc.scalar.dma_start(out=ysb[:, r0:r1, :], in_=ys[:, r0:r1, :])
            nc.sync.dma_start(out=wsb[:, 0], in_=wt[:, 0])
            nc.sync.dma_start(out=wsb[:, 1], in_=wt[:, 1])
            nc.sync.dma_start(out=bsb, in_=bb.ap())
            for r0, r1 in CHUNKS[1:]:
                nc.sync.dma_start(out=ysb[:, r0:r1, :], in_=ys[:, r0:r1, :])

            # uniform RB-row blocks: splitting the final block into smaller
            # pieces backfires -- each extra block adds a serialized
            # activation (+sem) to the post-train tail
            rblocks = [(rb, RB) for rb in range(0, HSH, RB)]
            for rb, nr in rblocks:
                for ob in range(2):
                    ps = pp.tile([O2, nr, W], F32, tag="ps", name=f"ps{rb}_{ob}")
                    for p in range(9):
                        i, j = divmod(p, 3)
                        nc.tensor.matmul(
                            ps,
                            wsb[:, ob, p],
                            ysb[:, rb + i : rb + i + nr, j : j + W],
                            start=(p == 0),
                            stop=(p == 8),
                        )
                    osb = op.tile([O2, nr, W], BF16, tag="osb", name=f"osb{rb}_{ob}")
                    nc.scalar.activation(
                        out=osb,
                        in_=ps,
                        func=ACT_IDENT,
                        bias=bsb[:, ob : ob + 1],
                        scale=1.0,
                    )
                    # per-ob output DMA: each half ships while the other
                    # half's matmuls still run, shortening the kernel tail
                    nc.sync.dma_start(
                        out=out[ob * O2 : (ob + 1) * O2, rb : rb + nr, :],
                        in_=osb,
                    )

    nc.compile()
    return nc


def _get_nc():
    if "nc" not in _CACHE:
        _CACHE["nc"] = build_nc()
    return _CACHE["nc"]


def make_in_maps(x, depth, weight, bias):
    x = np.asarray(x, np.float32)
    depth = np.asarray(depth, np.float32)
    weight = np.asarray(weight, np.float32)
    bias = np.asarray(bias, np.float32)
    y = x * depth  # (N, C, H, W) fp32; bf16-rounded below
    # (O, C, 3, 3) -> (C, ob, tap=i*3+j, o) with o = local index in the
    # 128-wide out-channel half ob
    wt9 = np.ascontiguousarray(
        np.transpose(weight.reshape(2, O2, C, 3, 3), (2, 0, 3, 4, 1)).reshape(
            C, 2, 9, O2
        )
    ).astype(NPBF16)
    bb = np.ascontiguousarray(bias.reshape(2, O2).T)
    in_maps = []
    for cid in range(NCORES):
        n, hh = divmod(cid, 2)
        ysh = np.zeros((C, HIN, W2), NPBF16)
        if hh == 0:
            ysh[:, 1:, 1 : W + 1] = y[n, :, : HSH + 1].astype(NPBF16)
        else:
            ysh[:, :-1, 1 : W + 1] = y[n, :, HSH - 1 :].astype(NPBF16)
        in_maps.append({"ys": ysh, "wt": wt9, "bb": bb})
    return in_maps


def gather_out(results):
    out = np.empty((N, O, H, W), np.float32)
    for cid in range(NCORES):
        n, hh = divmod(cid, 2)
        out[n, :, hh * HSH : (hh + 1) * HSH] = results[cid]["out"].astype(np.float32)
    return out


def kernel(x, depth, camera_params, weight, bias):
    nc = _get_nc()
    in_maps = make_in_maps(x, depth, weight, bias)
    res = run_bass_kernel_spmd(nc, in_maps, list(range(NCORES)))
    return gather_out(res.results)
